# revision 11
# baseline (speedup 1.0000x reference)
"""Cross-attention kernel for TRN2, 8-core SPMD.

Reference op (B=4, T=2048, S=512, D=1024, H=16, Hd=64):
    q = (x @ Wq + bq); k,v = context @ Wkv + bkv
    out = softmax(q k^T / sqrt(Hd) + mask) @ v @ Wp + bp

Sharding: pure data-parallel over (batch, T/2): core c owns batch c//2,
query rows (c%2)*1024..+1024.  Each core recomputes K/V for its batch
(2x duplicated KV-proj work, zero collectives).  Weights replicated.

v4 schedule (per core, R=1024 query rows).  The ACT Exp stream over the
score matrix (8.4M elements, ~90us at 128 lanes x 1.2GHz) is the pacing
engine for the attention inner loop, so everything is organized to
start it early and keep it dense:
  - head-streamed pipeline: k_proj(0..6) leads (only needs ctx+wk,
    ~3MB), then per head-pair q_proj -> QK -> Exp, with v_proj and
    av(hp-2) woven in; all independent PE work rides in the PE queue
    between exp-gated QK matmuls.
  - ALL psum tiles for projections and QK share one single-bank
    [128,512] pool (6 bufs) so the Exp stream is never throttled by
    psum recycling (a 2-buf double-bank QK pool serializes exp ->
    QK(s+1) -> exp at ~2.3x the exp cost).
  - input DMAs: each DGE queue sustains only ~150GB/s, so chunks are
    spread over the 3 queues (sync/SP, scalar/ACT, gpsimd) ordered by
    compute deadline; y output DMAs rotate over all 3 queues.
  - softmax denominators ride the AV matmuls for free (ones columns);
    sums are evicted to SBUF fp16 with a cross-partition DVE copy (the
    offset write aligns each head's sums with its O rows, verified on
    HW), one batched in-place ACT Reciprocal per rc-half (a single
    Exp->Recip LUT switch per kernel; v1 paid 12 table loads and a
    24us HAM re-throttle), recip(rc0) overlapping av(7,rc1) on the PE.
  - phase D (Y = OT^T @ Wp + bp) starts as soon as the rc0 rows are
    normalized; evictions + DMAs interleave; rc1 normalize hides under
    the first D row-blocks.

Layouts as v1: all activations transposed (feature on partitions), no
on-chip transposes; KT [D,S], QT [D,R] fp16; V_aug fp16 [S, 8x192] =
[V_even|ones|V_odd] with wv/ones pre-scaled 2^-10 so unnormalized
attention outputs fit fp16; QK contracts Hd=64 with two heads in PE
row-groups 0/64 concurrently; no max-subtraction (|scores| <= ~8).

Numerics: fp16 operands, fp32 PSUM accumulation.  Max-abs error vs the
fp32 reference ~1.1e-3 of max|out|.
"""
import os
import sys
import types

import numpy as np

import concourse.tile as tile
from concourse import bacc, mybir
from concourse.bass_utils import run_bass_kernel_spmd

F32 = mybir.dt.float32
F16 = mybir.dt.float16
AF = mybir.ActivationFunctionType

B, T, S, D = 4, 2048, 512, 1024
H, HD = 16, 64
NCORE = 8
R = B * T // NCORE          # 1024 query rows per core
KC = D // 128               # 8 contraction chunks
SC = S // 128               # 4 context chunks
NP = H // 2                 # 8 head pairs
NEG = -60.0                 # mask bias (exp(-60) ~ 0)

_CACHE = {}
last_results = None         # BassKernelResults of the most recent run


def _install_ntff_hook():
    """antenv.axon_hooks is absent in this image; recreate it from the
    boot helper so BASS_TRACE=1 profiling works. Best-effort."""
    try:
        import antenv.axon_hooks  # noqa: F401
        return
    except ImportError:
        pass
    try:
        from trn_agent_boot.trn_boot import _ntff_profile_via_ctypes
        hook = _ntff_profile_via_ctypes("/opt/axon/libaxon_pjrt.so")
        mod = types.ModuleType("antenv.axon_hooks")
        mod.get_axon_ntff_profile_hook = lambda: hook
        sys.modules["antenv.axon_hooks"] = mod
    except Exception:
        pass


_install_ntff_hook()


def _act_recip(nc, out_ap, in_ap):
    """Raw ACT Reciprocal (bass blocks the helper for accuracy reasons;
    measured ~1e-5 rel err here, plenty for softmax denominators)."""
    eng = nc.scalar
    return eng.add_instruction(
        mybir.InstActivation(
            name=nc.get_next_instruction_name(),
            func=AF.Reciprocal,
            ins=[eng.lower_ap(in_ap),
                 mybir.ImmediateValue(dtype=F32, value=0.0),
                 mybir.ImmediateValue(dtype=F32, value=1.0),
                 mybir.ImmediateValue(dtype=F32, value=0.0)],
            outs=[eng.lower_ap(out_ap)],
        ))


def _build():
    nc = bacc.Bacc("TRN2", target_bir_lowering=False, debug=False,
                   num_devices=NCORE)

    xT = nc.dram_tensor("xT", [D, R], F16, kind="ExternalInput").ap()
    ctxT = nc.dram_tensor("ctxT", [D, S], F16, kind="ExternalInput").ap()
    maskb = nc.dram_tensor("maskb", [128, SC], F32, kind="ExternalInput").ap()
    wq = nc.dram_tensor("wq", [D, D], F16, kind="ExternalInput").ap()
    bq = nc.dram_tensor("bq", [128, KC], F32, kind="ExternalInput").ap()
    wk = nc.dram_tensor("wk", [D, D], F16, kind="ExternalInput").ap()
    bk = nc.dram_tensor("bk", [128, KC], F32, kind="ExternalInput").ap()
    wv = nc.dram_tensor("wv", [D, D], F16, kind="ExternalInput").ap()
    wp = nc.dram_tensor("wp", [D, D], F16, kind="ExternalInput").ap()
    bp_r = nc.dram_tensor("bp_r", [128, D], F32, kind="ExternalInput").ap()
    ones = nc.dram_tensor("ones", [128, 512], F16, kind="ExternalInput").ap()
    y = nc.dram_tensor("y", [R, D], F32, kind="ExternalOutput").ap()

    with tile.TileContext(nc) as tc:
        # Pool stack bottom -> top; exp/psum pools on top so they close
        # after the attention stream, freeing room for psD / y.
        p_const = tc.tile_pool(name="const", bufs=1)
        p_kv = tc.tile_pool(name="kv", bufs=1)
        p_qt = tc.tile_pool(name="qt", bufs=1)
        p_ot = tc.tile_pool(name="ot", bufs=1)
        p_wp = tc.tile_pool(name="wpp", bufs=1)
        p_sums = tc.tile_pool(name="sums", bufs=1)
        p_ld = tc.tile_pool(name="ld", bufs=1)
        p_exp = tc.tile_pool(name="exp", bufs=44)
        p_ps = tc.tile_pool(name="ps", bufs=6, space="PSUM")
        p_psAV = tc.tile_pool(name="psAV", bufs=2, space="PSUM")
        constp = p_const.__enter__()
        kvp = p_kv.__enter__()
        qtp = p_qt.__enter__()
        otp = p_ot.__enter__()
        wpp = p_wp.__enter__()
        sumsp = p_sums.__enter__()
        ldp = p_ld.__enter__()
        expp = p_exp.__enter__()
        psp = p_ps.__enter__()
        psAV = p_psAV.__enter__()

        # ---- PE warm-up on a memset tile: covers the initial DMA
        # window and gets HAM to K=8/8 before real work ----
        warm_sb = constp.tile([128, 512], F16, tag="warm_sb")
        nc.vector.memset(warm_sb[:], 0.0)
        warm_ps = psp.tile([128, 512], F32, tag="ps")
        for w in range(20):
            nc.tensor.matmul(warm_ps[:], warm_sb[:, 0:128], warm_sb[:],
                             start=True, stop=True, skip_group_check=True)

        # ---- persistent tiles ----
        xT_t = [ldp.tile([128, R], F16, tag=f"xT{k}", name=f"xTs{k}")
                for k in range(KC)]
        wq_t = [ldp.tile([128, D], F16, tag=f"wq{k}", name=f"wqs{k}")
                for k in range(KC)]
        wk_t = [ldp.tile([128, D], F16, tag=f"wk{k}", name=f"wk{k}")
                for k in range(KC)]
        wv_t = [ldp.tile([128, D], F16, tag=f"wv{k}", name=f"wv{k}")
                for k in range(KC)]
        ctx_t = [ldp.tile([128, S], F16, tag=f"ctx{k}", name=f"ctx{k}")
                 for k in range(KC)]
        mb_t = constp.tile([128, SC], F32, tag="mb")
        bq_t = constp.tile([128, KC], F32, tag="bq")
        bk_t = constp.tile([128, KC], F32, tag="bk")
        bp_t = constp.tile([128, D], F32, tag="bp")
        wp_t = [wpp.tile([128, D], F16, tag=f"wp{k}", name=f"wps{k}")
                for k in range(KC)]
        KT = [kvp.tile([128, S], F16, tag=f"KT{m}", name=f"KT{m}")
              for m in range(KC)]
        # V_aug: [128, pair, 192] = [V_even | ones(64) | V_odd]
        VA = [kvp.tile([128, NP, 192], F16, tag=f"VA{s}", name=f"VA{s}")
              for s in range(SC)]
        QT = [qtp.tile([128, R], F16, tag=f"QT{m}", name=f"QT{m}")
              for m in range(KC)]
        OT = [otp.tile([128, R], F16, tag=f"OT{m}", name=f"OT{m}")
              for m in range(KC)]
        # softmax denominators, head-aligned with OT rows: [rc, hp]
        sums_t = sumsp.tile([128, 2, NP, 512], F16, tag="sums")

        # ---- input DMAs: ~150GB/s per DGE queue, so spread by compute
        # deadline: (wk,ctx) -> k_proj, then (wq,xT) -> q_proj, then wv,
        # wp, bp.  scalar's queue drains before the Exp stream starts.
        for k in range(4):
            nc.sync.dma_start(wk_t[k][:], wk[k * 128:(k + 1) * 128, :])
        for k in range(4):
            nc.sync.dma_start(wq_t[k][:], wq[k * 128:(k + 1) * 128, :])
        for s in range(SC):
            nc.sync.dma_start(VA[s][:, :, 64:128],
                              ones[:].rearrange("p (h c) -> p h c", c=64))
        for k in range(KC):
            nc.sync.dma_start(wp_t[k][:], wp[k * 128:(k + 1) * 128, :])
        nc.scalar.dma_start(bq_t[:], bq[:])
        nc.scalar.dma_start(bk_t[:], bk[:])
        nc.scalar.dma_start(mb_t[:], maskb[:])
        for k in range(KC):
            nc.scalar.dma_start(ctx_t[k][:], ctxT[k * 128:(k + 1) * 128, :])
        for k in range(KC):
            nc.scalar.dma_start(xT_t[k][:], xT[k * 128:(k + 1) * 128, :])
        for k in range(4, KC):
            nc.gpsimd.dma_start(wk_t[k][:], wk[k * 128:(k + 1) * 128, :])
        for k in range(4, KC):
            nc.gpsimd.dma_start(wq_t[k][:], wq[k * 128:(k + 1) * 128, :])
        for k in range(KC):
            nc.gpsimd.dma_start(wv_t[k][:], wv[k * 128:(k + 1) * 128, :])
        nc.gpsimd.dma_start(bp_t[:], bp_r[:])

        def q_proj(m):
            for rc in range(2):
                ps = psp.tile([128, 512], F32, tag="ps")
                for k in range(KC):
                    nc.tensor.matmul(
                        ps[:], wq_t[k][:, m * 128:(m + 1) * 128],
                        xT_t[k][:, rc * 512:(rc + 1) * 512],
                        start=(k == 0), stop=(k == KC - 1))
                nc.vector.tensor_scalar_add(
                    QT[m][:, rc * 512:(rc + 1) * 512], ps[:], bq_t[:, m:m + 1])

        def k_proj(m):
            ps = psp.tile([128, S], F32, tag="ps")
            for k in range(KC):
                nc.tensor.matmul(ps[:], wk_t[k][:, m * 128:(m + 1) * 128],
                                 ctx_t[k][:],
                                 start=(k == 0), stop=(k == KC - 1))
            nc.vector.tensor_scalar_add(KT[m][:], ps[:], bk_t[:, m:m + 1])

        def v_proj(n):
            for s in range(SC):
                ps = psp.tile([128, 512], F32, tag="ps")
                for k in range(KC):
                    nc.tensor.matmul(ps[:], ctx_t[k][:, s * 128:(s + 1) * 128],
                                     wv_t[k][:, n * 512:(n + 1) * 512],
                                     start=(k == 0), stop=(k == KC - 1))
                # scatter 8 heads (4 pairs) into V_aug blocks
                src = ps[:].rearrange("p (h c) -> p h c", c=64)
                nc.vector.tensor_copy(VA[s][:, 4 * n:4 * n + 4, 0:64],
                                      src[:, 0::2, :])
                nc.vector.tensor_copy(VA[s][:, 4 * n:4 * n + 4, 128:192],
                                      src[:, 1::2, :])

        def attn_qk(hp):
            # ex[e][rc][s]: single-bank psum per score block so the ACT
            # Exp stream never waits on psum recycling
            ex = [[[expp.tile([128, 512], F16, tag="exp",
                              name=f"ex{hp}_{e}_{rc}_{s}")
                    for s in range(SC)] for rc in range(2)] for e in range(2)]
            for s in range(SC):
                for rc in range(2):
                    pss = []
                    for e in range(2):
                        # two head row-groups run concurrently in the PE
                        lo, hi = 64 * e, 64 * e + 64
                        ps = psp.tile([128, 512], F32, tag="ps",
                                      name=f"psqk{hp}_{s}_{rc}_{e}")
                        nc.tensor.matmul(
                            ps[:],
                            KT[hp][lo:hi, s * 128:(s + 1) * 128],
                            QT[hp][lo:hi, rc * 512:(rc + 1) * 512],
                            start=True, stop=True)
                        pss.append(ps)
                    for e in range(2):
                        nc.scalar.activation(ex[e][rc][s][:], pss[e][:],
                                             AF.Exp, bias=mb_t[:, s:s + 1])
            return ex

        def attn_av(hp, ex, rcs=(0, 1)):
            for rc in rcs:
                rr = slice(rc * 512, rc * 512 + 512)
                for e in range(2):
                    # even head: V cols 0:128 -> O rows 0:64, sums 64:128
                    # odd  head: V cols 64:192 -> sums 0:64, O rows 64:128
                    voff = 64 * e
                    olo, ohi = (0, 64) if e == 0 else (64, 128)
                    slo, shi = (64, 128) if e == 0 else (0, 64)
                    ps = psAV.tile([128, 512], F32, tag="psAV")
                    for s in range(SC):
                        nc.tensor.matmul(
                            ps[:], VA[s][:, hp, voff:voff + 128],
                            ex[e][rc][s][:],
                            start=(s == 0), stop=(s == SC - 1))
                    nc.vector.tensor_copy(OT[hp][olo:ohi, rr],
                                          ps[olo:ohi, :])
                    # cross-partition evict: head hp's sums land in the
                    # opposite 64-half; write them aligned with its O
                    nc.vector.tensor_copy(sums_t[olo:ohi, rc, hp, :],
                                          ps[slo:shi, :])

        # ============ head-streamed pipeline ============
        pending_ex = {}
        for m in range(7):
            k_proj(m)
        for hp in range(NP):
            q_proj(hp)
            if hp == 1:
                k_proj(7)
            if hp == 2:
                v_proj(0)
            if hp == 4:
                v_proj(1)
            pending_ex[hp] = attn_qk(hp)
            if hp >= 2:
                attn_av(hp - 2, pending_ex.pop(hp - 2))
        attn_av(NP - 2, pending_ex.pop(NP - 2))
        ex7 = pending_ex.pop(NP - 1)
        attn_av(NP - 1, ex7, rcs=(0,))
        # recip rc0 on ACT overlaps av(7, rc1) on the PE
        _act_recip(nc, sums_t[:, 0, :, :], sums_t[:, 0, :, :])
        attn_av(NP - 1, ex7, rcs=(1,))
        for hp in range(NP):
            nc.vector.tensor_mul(OT[hp][:, 0:512], OT[hp][:, 0:512],
                                 sums_t[:, 0, hp, :])
        _act_recip(nc, sums_t[:, 1, :, :], sums_t[:, 1, :, :])
        for hp in range(NP):
            nc.vector.tensor_mul(OT[hp][:, 512:1024], OT[hp][:, 512:1024],
                                 sums_t[:, 1, hp, :])

        p_psAV.__exit__(None, None, None)
        p_ps.__exit__(None, None, None)
        p_exp.__exit__(None, None, None)

        # ================= output projection =================
        p_psD = tc.tile_pool(name="psD", bufs=5, space="PSUM")
        psD = p_psD.__enter__()
        p_y = tc.tile_pool(name="y", bufs=4)
        yp = p_y.__enter__()
        dma_engs = [nc.sync, nc.gpsimd, nc.scalar]
        for rp in range(KC):
            for n in range(2):
                ps = psD.tile([128, 512], F32, tag="psD")
                for k in range(KC):
                    nc.tensor.matmul(
                        ps[:], OT[k][:, rp * 128:(rp + 1) * 128],
                        wp_t[k][:, n * 512:(n + 1) * 512],
                        start=(k == 0), stop=(k == KC - 1))
                yt = yp.tile([128, 512], F32, tag="y")
                nc.vector.tensor_add(yt[:], ps[:], bp_t[:, n * 512:(n + 1) * 512])
                eng = dma_engs[(rp * 2 + n) % 3]
                eng.dma_start(
                    y[rp * 128:(rp + 1) * 128, n * 512:(n + 1) * 512], yt[:])
        p_y.__exit__(None, None, None)
        p_psD.__exit__(None, None, None)
        p_ld.__exit__(None, None, None)
        p_sums.__exit__(None, None, None)
        p_wp.__exit__(None, None, None)
        p_ot.__exit__(None, None, None)
        p_qt.__exit__(None, None, None)
        p_kv.__exit__(None, None, None)
        p_const.__exit__(None, None, None)

    nc.compile()
    return nc


def _get_nc():
    if "nc" not in _CACHE:
        _CACHE["nc"] = _build()
    return _CACHE["nc"]


def kernel(x, context, context_mask, Wq, bq, Wkv, bkv, Wp, bp):
    global last_results
    x = np.asarray(x, dtype=np.float32)
    context = np.asarray(context, dtype=np.float32)
    context_mask = np.asarray(context_mask)
    Wq = np.asarray(Wq, dtype=np.float32)
    bq = np.asarray(bq, dtype=np.float32)
    Wkv = np.asarray(Wkv, dtype=np.float32)
    bkv = np.asarray(bkv, dtype=np.float32)
    Wp = np.asarray(Wp, dtype=np.float32)
    bp = np.asarray(bp, dtype=np.float32)

    sc = 1.0 / np.sqrt(HD)
    # kv reshape in the reference is [S, 2, H, Hd]: k cols = Wkv[:, :D]
    wq_h = np.ascontiguousarray((Wq * sc).astype(np.float16))
    bq_h = np.ascontiguousarray((bq * sc).reshape(KC, 128).T)
    wk_h = np.ascontiguousarray(Wkv[:, :D].astype(np.float16))
    bk_h = np.ascontiguousarray(bkv[:D].reshape(KC, 128).T)
    wv_h = np.ascontiguousarray((Wkv[:, D:] * 2.0**-10).astype(np.float16))
    bv = bkv[D:]
    wp_h = np.ascontiguousarray(Wp.astype(np.float16))
    bp_eff = bp + bv @ Wp          # softmax rows sum to 1
    bp_r = np.ascontiguousarray(
        np.broadcast_to(bp_eff.astype(np.float32), (128, D)))
    ones_h = np.full((128, 512), 2.0**-10, dtype=np.float16)

    in_maps = []
    for c in range(NCORE):
        b = c // 2
        r0 = (c % 2) * R
        in_maps.append({
            "xT": np.ascontiguousarray(x[b, r0:r0 + R, :].T.astype(np.float16)),
            "ctxT": np.ascontiguousarray(context[b].T.astype(np.float16)),
            "maskb": np.ascontiguousarray(
                np.where(context_mask[b], 0.0, NEG).astype(np.float32)
                .reshape(SC, 128).T),
            "wq": wq_h, "bq": bq_h,
            "wk": wk_h, "bk": bk_h,
            "wv": wv_h,
            "wp": wp_h, "bp_r": bp_r, "ones": ones_h,
        })

    nc = _get_nc()
    res = run_bass_kernel_spmd(nc, in_maps, list(range(NCORE)),
                               trace=bool(os.environ.get("BASS_TRACE")))
    last_results = res

    out = np.empty((B, T, D), dtype=np.float32)
    for c in range(NCORE):
        b = c // 2
        r0 = (c % 2) * R
        out[b, r0:r0 + R, :] = res.results[c]["y"]
    return out


# revision 12
# speedup vs baseline: 1.1329x; 1.1329x over previous
"""Cross-attention kernel for TRN2, 8-core SPMD.

Reference op (B=4, T=2048, S=512, D=1024, H=16, Hd=64):
    q = (x @ Wq + bq); k,v = context @ Wkv + bkv
    out = softmax(q k^T / sqrt(Hd) + mask) @ v @ Wp + bp

Sharding: pure data-parallel over (batch, T/2): core c owns batch c//2,
query rows (c%2)*1024..+1024.  Each core recomputes K/V for its batch
(2x duplicated KV-proj work, zero collectives).  Weights replicated.

v4 schedule (per core, R=1024 query rows).  The ACT Exp stream over the
score matrix (8.4M elements, ~90us at 128 lanes x 1.2GHz) is the pacing
engine for the attention inner loop, so everything is organized to
start it early and keep it dense:
  - head-streamed pipeline: k_proj(0..6) leads (only needs ctx+wk,
    ~3MB), then per head-pair q_proj -> QK -> Exp, with v_proj and
    av(hp-2) woven in; all independent PE work rides in the PE queue
    between exp-gated QK matmuls.
  - ALL psum tiles for projections and QK share one single-bank
    [128,512] pool (6 bufs) so the Exp stream is never throttled by
    psum recycling (a 2-buf double-bank QK pool serializes exp ->
    QK(s+1) -> exp at ~2.3x the exp cost).
  - input DMAs: each DGE queue sustains only ~150GB/s, so chunks are
    spread over the 3 queues (sync/SP, scalar/ACT, gpsimd) ordered by
    compute deadline; y output DMAs rotate over all 3 queues.
  - softmax denominators ride the AV matmuls for free (ones columns);
    sums are evicted to SBUF fp16 with a cross-partition DVE copy (the
    offset write aligns each head's sums with its O rows, verified on
    HW), one batched in-place ACT Reciprocal per rc-half (a single
    Exp->Recip LUT switch per kernel; v1 paid 12 table loads and a
    24us HAM re-throttle), recip(rc0) overlapping av(7,rc1) on the PE.
  - phase D (Y = OT^T @ Wp + bp) starts as soon as the rc0 rows are
    normalized; evictions + DMAs interleave; rc1 normalize hides under
    the first D row-blocks.

Layouts as v1: all activations transposed (feature on partitions), no
on-chip transposes; KT [D,S], QT [D,R] fp16; V_aug fp16 [S, 8x192] =
[V_even|ones|V_odd] with wv/ones pre-scaled 2^-10 so unnormalized
attention outputs fit fp16; QK contracts Hd=64 with two heads in PE
row-groups 0/64 concurrently; no max-subtraction (|scores| <= ~8).

Numerics: fp16 operands, fp32 PSUM accumulation.  Max-abs error vs the
fp32 reference ~1.1e-3 of max|out|.
"""
import os
import sys
import types

import numpy as np

import concourse.tile as tile
from concourse import bacc, mybir
from concourse.bass_utils import run_bass_kernel_spmd

F32 = mybir.dt.float32
F16 = mybir.dt.float16
AF = mybir.ActivationFunctionType

B, T, S, D = 4, 2048, 512, 1024
H, HD = 16, 64
NCORE = 8
R = B * T // NCORE          # 1024 query rows per core
KC = D // 128               # 8 contraction chunks
SC = S // 128               # 4 context chunks
NP = H // 2                 # 8 head pairs
NEG = -60.0                 # mask bias (exp(-60) ~ 0)

_CACHE = {}
last_results = None         # BassKernelResults of the most recent run


def _install_ntff_hook():
    """antenv.axon_hooks is absent in this image; recreate it from the
    boot helper so BASS_TRACE=1 profiling works. Best-effort."""
    try:
        import antenv.axon_hooks  # noqa: F401
        return
    except ImportError:
        pass
    try:
        from trn_agent_boot.trn_boot import _ntff_profile_via_ctypes
        hook = _ntff_profile_via_ctypes("/opt/axon/libaxon_pjrt.so")
        mod = types.ModuleType("antenv.axon_hooks")
        mod.get_axon_ntff_profile_hook = lambda: hook
        sys.modules["antenv.axon_hooks"] = mod
    except Exception:
        pass


_install_ntff_hook()


def _act_recip(nc, out_ap, in_ap):
    """Raw ACT Reciprocal (bass blocks the helper for accuracy reasons;
    measured ~1e-5 rel err here, plenty for softmax denominators)."""
    eng = nc.scalar
    return eng.add_instruction(
        mybir.InstActivation(
            name=nc.get_next_instruction_name(),
            func=AF.Reciprocal,
            ins=[eng.lower_ap(in_ap),
                 mybir.ImmediateValue(dtype=F32, value=0.0),
                 mybir.ImmediateValue(dtype=F32, value=1.0),
                 mybir.ImmediateValue(dtype=F32, value=0.0)],
            outs=[eng.lower_ap(out_ap)],
        ))


def _build():
    nc = bacc.Bacc("TRN2", target_bir_lowering=False, debug=False,
                   num_devices=NCORE)

    xT = nc.dram_tensor("xT", [D, R], F16, kind="ExternalInput").ap()
    ctxT = nc.dram_tensor("ctxT", [D, S], F16, kind="ExternalInput").ap()
    maskb = nc.dram_tensor("maskb", [128, SC], F32, kind="ExternalInput").ap()
    wq = nc.dram_tensor("wq", [D, D], F16, kind="ExternalInput").ap()
    bq = nc.dram_tensor("bq", [128, KC], F32, kind="ExternalInput").ap()
    wk = nc.dram_tensor("wk", [D, D], F16, kind="ExternalInput").ap()
    bk = nc.dram_tensor("bk", [128, KC], F32, kind="ExternalInput").ap()
    wv = nc.dram_tensor("wv", [D, D], F16, kind="ExternalInput").ap()
    wp = nc.dram_tensor("wp", [D, D], F16, kind="ExternalInput").ap()
    bp_r = nc.dram_tensor("bp_r", [128, D], F32, kind="ExternalInput").ap()
    ones = nc.dram_tensor("ones", [128, 512], F16, kind="ExternalInput").ap()
    y = nc.dram_tensor("y", [R, D], F32, kind="ExternalOutput").ap()

    with tile.TileContext(nc) as tc:
        # Pool stack bottom -> top; exp/psum pools on top so they close
        # after the attention stream, freeing room for psD / y.
        p_const = tc.tile_pool(name="const", bufs=1)
        p_kv = tc.tile_pool(name="kv", bufs=1)
        p_qt = tc.tile_pool(name="qt", bufs=1)
        p_ot = tc.tile_pool(name="ot", bufs=1)
        p_wp = tc.tile_pool(name="wpp", bufs=1)
        p_sums = tc.tile_pool(name="sums", bufs=1)
        p_ld = tc.tile_pool(name="ld", bufs=1)
        p_exp = tc.tile_pool(name="exp", bufs=44)
        p_ps = tc.tile_pool(name="ps", bufs=6, space="PSUM")
        p_psAV = tc.tile_pool(name="psAV", bufs=2, space="PSUM")
        constp = p_const.__enter__()
        kvp = p_kv.__enter__()
        qtp = p_qt.__enter__()
        otp = p_ot.__enter__()
        wpp = p_wp.__enter__()
        sumsp = p_sums.__enter__()
        ldp = p_ld.__enter__()
        expp = p_exp.__enter__()
        psp = p_ps.__enter__()
        psAV = p_psAV.__enter__()

        # ---- PE warm-up on a memset tile: covers the initial DMA
        # window and gets HAM to K=8/8 before real work ----
        warm_sb = constp.tile([128, 512], F16, tag="warm_sb")
        nc.vector.memset(warm_sb[:], 0.0)
        warm_ps = psp.tile([128, 512], F32, tag="ps")
        for w in range(20):
            nc.tensor.matmul(warm_ps[:], warm_sb[:, 0:128], warm_sb[:],
                             start=True, stop=True, skip_group_check=True)

        # ---- persistent tiles ----
        xT_t = [ldp.tile([128, R], F16, tag=f"xT{k}", name=f"xTs{k}")
                for k in range(KC)]
        wq_t = [ldp.tile([128, D], F16, tag=f"wq{k}", name=f"wqs{k}")
                for k in range(KC)]
        wk_t = [ldp.tile([128, D], F16, tag=f"wk{k}", name=f"wk{k}")
                for k in range(KC)]
        wv_t = [ldp.tile([128, D], F16, tag=f"wv{k}", name=f"wv{k}")
                for k in range(KC)]
        ctx_t = [ldp.tile([128, S], F16, tag=f"ctx{k}", name=f"ctx{k}")
                 for k in range(KC)]
        mb_t = constp.tile([128, SC], F32, tag="mb")
        bq_t = constp.tile([128, KC], F32, tag="bq")
        bk_t = constp.tile([128, KC], F32, tag="bk")
        bp_t = constp.tile([128, D], F32, tag="bp")
        wp_t = [wpp.tile([128, D], F16, tag=f"wp{k}", name=f"wps{k}")
                for k in range(KC)]
        KT = [kvp.tile([128, S], F16, tag=f"KT{m}", name=f"KT{m}")
              for m in range(KC)]
        # V_aug: [128, pair, 192] = [V_even | ones(64) | V_odd]
        VA = [kvp.tile([128, NP, 192], F16, tag=f"VA{s}", name=f"VA{s}")
              for s in range(SC)]
        QT = [qtp.tile([128, R], F16, tag=f"QT{m}", name=f"QT{m}")
              for m in range(KC)]
        OT = [otp.tile([128, R], F16, tag=f"OT{m}", name=f"OT{m}")
              for m in range(KC)]
        # softmax denominators, head-aligned with OT rows: [rc, hp]
        sums_t = sumsp.tile([128, 2, NP, 512], F16, tag="sums")

        # ---- input DMAs: ~150GB/s per DGE queue, so spread by compute
        # deadline: (wk,ctx) -> k_proj, then (wq,xT) -> q_proj, then wv,
        # wp, bp.  scalar's queue drains before the Exp stream starts.
        for k in range(4):
            nc.sync.dma_start(wk_t[k][:], wk[k * 128:(k + 1) * 128, :])
        for k in range(4):
            nc.sync.dma_start(wq_t[k][:], wq[k * 128:(k + 1) * 128, :])
        for s in range(SC):
            nc.sync.dma_start(VA[s][:, :, 64:128],
                              ones[:].rearrange("p (h c) -> p h c", c=64))
        for k in range(KC):
            nc.sync.dma_start(wp_t[k][:], wp[k * 128:(k + 1) * 128, :])
        nc.scalar.dma_start(bq_t[:], bq[:])
        nc.scalar.dma_start(bk_t[:], bk[:])
        nc.scalar.dma_start(mb_t[:], maskb[:])
        for k in range(KC):
            nc.scalar.dma_start(ctx_t[k][:], ctxT[k * 128:(k + 1) * 128, :])
        for k in range(KC):
            nc.scalar.dma_start(xT_t[k][:], xT[k * 128:(k + 1) * 128, :])
        for k in range(4, KC):
            nc.gpsimd.dma_start(wk_t[k][:], wk[k * 128:(k + 1) * 128, :])
        for k in range(4, KC):
            nc.gpsimd.dma_start(wq_t[k][:], wq[k * 128:(k + 1) * 128, :])
        for k in range(KC):
            nc.gpsimd.dma_start(wv_t[k][:], wv[k * 128:(k + 1) * 128, :])
        nc.gpsimd.dma_start(bp_t[:], bp_r[:])

        def q_proj(m, rc):
            ps = psp.tile([128, 512], F32, tag="ps")
            for k in range(KC):
                nc.tensor.matmul(
                    ps[:], wq_t[k][:, m * 128:(m + 1) * 128],
                    xT_t[k][:, rc * 512:(rc + 1) * 512],
                    start=(k == 0), stop=(k == KC - 1))
            nc.vector.tensor_scalar_add(
                QT[m][:, rc * 512:(rc + 1) * 512], ps[:], bq_t[:, m:m + 1])

        def k_proj(m):
            ps = psp.tile([128, S], F32, tag="ps")
            for k in range(KC):
                nc.tensor.matmul(ps[:], wk_t[k][:, m * 128:(m + 1) * 128],
                                 ctx_t[k][:],
                                 start=(k == 0), stop=(k == KC - 1))
            nc.vector.tensor_scalar_add(KT[m][:], ps[:], bk_t[:, m:m + 1])

        def v_proj(n, s):
            ps = psp.tile([128, 512], F32, tag="ps")
            for k in range(KC):
                nc.tensor.matmul(ps[:], ctx_t[k][:, s * 128:(s + 1) * 128],
                                 wv_t[k][:, n * 512:(n + 1) * 512],
                                 start=(k == 0), stop=(k == KC - 1))
            # scatter 8 heads (4 pairs) into V_aug blocks
            vsrc = ps[:].rearrange("p (h c) -> p h c", c=64)
            nc.vector.tensor_copy(VA[s][:, 4 * n:4 * n + 4, 0:64],
                                  vsrc[:, 0::2, :])
            nc.vector.tensor_copy(VA[s][:, 4 * n:4 * n + 4, 128:192],
                                  vsrc[:, 1::2, :])

        def qk_slot(hp, ex, rc, s):
            """One (rc, s) score block: 2 concurrent row-group matmuls
            + 2 Exps.  Fill work is emitted between slots so the PE
            FIFO never head-of-line-blocks the ACT Exp stream."""
            pss = []
            for e in range(2):
                lo, hi = 64 * e, 64 * e + 64
                ps = psp.tile([128, 512], F32, tag="ps",
                              name=f"psqk{hp}_{s}_{rc}_{e}")
                nc.tensor.matmul(
                    ps[:],
                    KT[hp][lo:hi, s * 128:(s + 1) * 128],
                    QT[hp][lo:hi, rc * 512:(rc + 1) * 512],
                    start=True, stop=True)
                pss.append(ps)
            for e in range(2):
                nc.scalar.activation(ex[e][rc][s][:], pss[e][:],
                                     AF.Exp, bias=mb_t[:, s:s + 1])

        def attn_av(hp, ex, rc):
            rr = slice(rc * 512, rc * 512 + 512)
            for e in range(2):
                # even head: V cols 0:128 -> O rows 0:64, sums 64:128
                # odd  head: V cols 64:192 -> sums 0:64, O rows 64:128
                voff = 64 * e
                olo, ohi = (0, 64) if e == 0 else (64, 128)
                slo, shi = (64, 128) if e == 0 else (0, 64)
                ps = psAV.tile([128, 512], F32, tag="psAV")
                for s in range(SC):
                    nc.tensor.matmul(
                        ps[:], VA[s][:, hp, voff:voff + 128],
                        ex[e][rc][s][:],
                        start=(s == 0), stop=(s == SC - 1))
                nc.vector.tensor_copy(OT[hp][olo:ohi, rr],
                                      ps[olo:ohi, :])
                # cross-partition evict: head hp's sums land in the
                # opposite 64-half; write them aligned with its O
                nc.vector.tensor_copy(sums_t[olo:ohi, rc, hp, :],
                                      ps[slo:shi, :])

        # ============ head-streamed pipeline ============
        # q_proj halves (qc), v_proj chains (vc) and av pairs (ac) are
        # dribbled between the (rc, s) score slots from this ordered
        # work list, so neither the PE FIFO nor the Exp stream ever
        # stalls the other for more than ~1us.
        def qc(m, rc):
            return lambda ex: q_proj(m, rc)

        def vc(n, s):
            return lambda ex: v_proj(n, s)

        def ac(hp, rc):
            return lambda ex: attn_av(hp, ex[hp], rc)

        work = [
            qc(0, 1), qc(1, 0), qc(1, 1), qc(2, 0), vc(0, 0), qc(2, 1),
            vc(0, 1), vc(0, 2), qc(3, 0), vc(0, 3), qc(3, 1), ac(0, 0),
            qc(4, 0), ac(0, 1), qc(4, 1), ac(1, 0), qc(5, 0), ac(1, 1),
            qc(5, 1), ac(2, 0), qc(6, 0), ac(2, 1), qc(6, 1), vc(1, 0),
            ac(3, 0), qc(7, 0), vc(1, 1), ac(3, 1), qc(7, 1), vc(1, 2),
            vc(1, 3), ac(4, 0), ac(4, 1), ac(5, 0), ac(5, 1), ac(6, 0),
            ac(6, 1),
        ]
        for m in range(KC):
            k_proj(m)
        q_proj(0, 0)
        exs = {}
        slot = 0
        popped = 0
        for hp in range(NP):
            exs[hp] = [[[expp.tile([128, 512], F16, tag="exp",
                                   name=f"ex{hp}_{e}_{rc}_{s}")
                         for s in range(SC)] for rc in range(2)]
                       for e in range(2)]
            for rc in range(2):
                for s in range(SC):
                    qk_slot(hp, exs[hp], rc, s)
                    slot += 1
                    want = (slot * len(work)) // (NP * 2 * SC)
                    while popped < want:
                        work[popped](exs)
                        popped += 1
        while popped < len(work):
            work[popped](exs)
            popped += 1
        attn_av(NP - 1, exs[NP - 1], 0)
        # recip rc0 on ACT overlaps av(7, rc1) on the PE
        _act_recip(nc, sums_t[:, 0, :, :], sums_t[:, 0, :, :])
        attn_av(NP - 1, exs[NP - 1], 1)
        for hp in range(NP):
            nc.vector.tensor_mul(OT[hp][:, 0:512], OT[hp][:, 0:512],
                                 sums_t[:, 0, hp, :])
        _act_recip(nc, sums_t[:, 1, :, :], sums_t[:, 1, :, :])
        for hp in range(NP):
            nc.vector.tensor_mul(OT[hp][:, 512:1024], OT[hp][:, 512:1024],
                                 sums_t[:, 1, hp, :])

        p_psAV.__exit__(None, None, None)
        p_ps.__exit__(None, None, None)
        p_exp.__exit__(None, None, None)

        # ================= output projection =================
        p_psD = tc.tile_pool(name="psD", bufs=5, space="PSUM")
        psD = p_psD.__enter__()
        p_y = tc.tile_pool(name="y", bufs=4)
        yp = p_y.__enter__()
        dma_engs = [nc.sync, nc.gpsimd, nc.scalar]
        for rp in range(KC):
            for n in range(2):
                ps = psD.tile([128, 512], F32, tag="psD")
                for k in range(KC):
                    nc.tensor.matmul(
                        ps[:], OT[k][:, rp * 128:(rp + 1) * 128],
                        wp_t[k][:, n * 512:(n + 1) * 512],
                        start=(k == 0), stop=(k == KC - 1))
                yt = yp.tile([128, 512], F32, tag="y")
                nc.vector.tensor_add(yt[:], ps[:], bp_t[:, n * 512:(n + 1) * 512])
                eng = dma_engs[(rp * 2 + n) % 3]
                eng.dma_start(
                    y[rp * 128:(rp + 1) * 128, n * 512:(n + 1) * 512], yt[:])
        p_y.__exit__(None, None, None)
        p_psD.__exit__(None, None, None)
        p_ld.__exit__(None, None, None)
        p_sums.__exit__(None, None, None)
        p_wp.__exit__(None, None, None)
        p_ot.__exit__(None, None, None)
        p_qt.__exit__(None, None, None)
        p_kv.__exit__(None, None, None)
        p_const.__exit__(None, None, None)

    nc.compile()
    return nc


def _get_nc():
    if "nc" not in _CACHE:
        _CACHE["nc"] = _build()
    return _CACHE["nc"]


def kernel(x, context, context_mask, Wq, bq, Wkv, bkv, Wp, bp):
    global last_results
    x = np.asarray(x, dtype=np.float32)
    context = np.asarray(context, dtype=np.float32)
    context_mask = np.asarray(context_mask)
    Wq = np.asarray(Wq, dtype=np.float32)
    bq = np.asarray(bq, dtype=np.float32)
    Wkv = np.asarray(Wkv, dtype=np.float32)
    bkv = np.asarray(bkv, dtype=np.float32)
    Wp = np.asarray(Wp, dtype=np.float32)
    bp = np.asarray(bp, dtype=np.float32)

    sc = 1.0 / np.sqrt(HD)
    # kv reshape in the reference is [S, 2, H, Hd]: k cols = Wkv[:, :D]
    wq_h = np.ascontiguousarray((Wq * sc).astype(np.float16))
    bq_h = np.ascontiguousarray((bq * sc).reshape(KC, 128).T)
    wk_h = np.ascontiguousarray(Wkv[:, :D].astype(np.float16))
    bk_h = np.ascontiguousarray(bkv[:D].reshape(KC, 128).T)
    wv_h = np.ascontiguousarray((Wkv[:, D:] * 2.0**-10).astype(np.float16))
    bv = bkv[D:]
    wp_h = np.ascontiguousarray(Wp.astype(np.float16))
    bp_eff = bp + bv @ Wp          # softmax rows sum to 1
    bp_r = np.ascontiguousarray(
        np.broadcast_to(bp_eff.astype(np.float32), (128, D)))
    ones_h = np.full((128, 512), 2.0**-10, dtype=np.float16)

    in_maps = []
    for c in range(NCORE):
        b = c // 2
        r0 = (c % 2) * R
        in_maps.append({
            "xT": np.ascontiguousarray(x[b, r0:r0 + R, :].T.astype(np.float16)),
            "ctxT": np.ascontiguousarray(context[b].T.astype(np.float16)),
            "maskb": np.ascontiguousarray(
                np.where(context_mask[b], 0.0, NEG).astype(np.float32)
                .reshape(SC, 128).T),
            "wq": wq_h, "bq": bq_h,
            "wk": wk_h, "bk": bk_h,
            "wv": wv_h,
            "wp": wp_h, "bp_r": bp_r, "ones": ones_h,
        })

    nc = _get_nc()
    res = run_bass_kernel_spmd(nc, in_maps, list(range(NCORE)),
                               trace=bool(os.environ.get("BASS_TRACE")))
    last_results = res

    out = np.empty((B, T, D), dtype=np.float32)
    for c in range(NCORE):
        b = c // 2
        r0 = (c % 2) * R
        out[b, r0:r0 + R, :] = res.results[c]["y"]
    return out
